# revision 16
# baseline (speedup 1.0000x reference)
"""Distributed Trainium2 (Bass/Tile) kernel for the contrastive loss.

Strategy (8 NeuronCores, SPMD, row-sharded similarity matrix):
  Core c owns 512 of the 4096 rows of sim = reps @ reps^T (per l).
  Host rolls the column order by c*512 so one NEFF serves all cores.

  Layout trick: host feeds raw embeddings already TRANSPOSED ([D, row]
  bf16), so the big matrix is never transposed on device.  Sim tiles
  are computed "transposed": out[n, m] = x_n . z_m with n = all 4096
  raw rows on partitions and m = 512 normalized local rows on the free
  axis.  The n-side normalization folds into the exp stage as a
  per-partition factor: exp(sim/T) = exp((inv_n/T) * G[n,m]).

  The exp work is split across the two engines that can read PSUM:
    - ACT: activation(Exp, scale=inv_n/T per partition) -> bf16 E
    - DVE: Schraudolph float-bits exp: one tensor_scalar
      (G * [inv_n/T * 2^23/ln2] + B) written to int32; the int bits
      reinterpreted as f32 are e^(sim/T) to within ~4% per entry
      (mean bias ~2e-4, well inside the denominator's error budget).
  GPSIMD cannot touch PSUM on TRN2, so Pool instead runs the SBUF-side
  scalar work (local-row scaling, self-clamp, inv scaling, tail).

  Per-row denominators are column sums of E via near-free stationary
  matmuls (lhsT = E chunk [128,128], rhs = ones -> [128,1] out,
  ap_size=1) accumulated in PSUM across all 32 n-chunks; the exp/sums
  for chunk j+1..j+7 overlap the colsums of chunk j.

  The self-similarity entries (exp(z.z/T) ~ e^5 ~ 148 vs cross terms
  < ~17) are clamped to exactly 64 in E and subtracted exactly via the
  Ln bias.  The positive-pair term is recovered per row as
  ln(E_diag) * inv_m.  Norms: ssq via DVE square + 32 stationary
  matmuls against ones per l; inv = exp(-0.5 ln ssq) on ACT.  The
  per-l prologue is software-pipelined into the previous l's sim loop.

  Host sums the 8 output tensors -> scalar loss (the all-reduce).
"""

import numpy as np

TEMP = 0.2
INV_T = 1.0 / TEMP
L, B, K, D = 4, 64, 32, 128
N = B * K          # 2048
M = 2 * N          # 4096 rows of sim per l
NCORES = 8
R = M // NCORES    # 512 local rows per core
MS = 4             # m sub-blocks of 128
NCH = 32           # n-chunks of 128 per l
NT = 16            # sim tiles of [128, 2, 512] per l (2 chunks each)
CLAMP = 16384.0    # exactly representable in bf16; >> e^2.7, << e^40

# Exp engines (real-HW legal set): ACT Exp reads PSUM directly; DVE has no
# pow/exp, so DVE chunks use the Schraudolph float-bits trick -- one
# tensor_scalar (mult by per-partition A_n = inv_n/T * 2^23/ln2, add B)
# writing int32; the int bits ARE the bf-pattern of exp. GPSIMD cannot read
# PSUM at all, so Pool only gets SBUF-side scalar work.
SCH_S = 12102203.161561485          # 2^23 / ln 2
SCH_B = 1064868216.0                # 127*2^23 - 485000 (calibrated)

def _mk_sched():
    forced_a = {0, 1, 2, 3, 16, 17, 18, 19}
    pat = ["D", "D", "A", "D", "A", "D", "D", "A"]
    items, k = [], 0
    for j in range(32):
        if j in forced_a:
            items.append(("A", j))
        else:
            items.append((pat[k % 8], j)); k += 1
    return items

SCHED = _mk_sched()

_built = None


def _build():
    global _built
    if _built is not None:
        return _built
    from contextlib import ExitStack

    import concourse.tile as tile
    from concourse import bacc
    import concourse.mybir as mybir
    from concourse.masks import make_identity

    f32 = mybir.dt.float32
    bf16 = mybir.dt.bfloat16
    i32 = mybir.dt.int32
    AF = mybir.ActivationFunctionType
    OP = mybir.AluOpType

    # Pin every ACT op to the natural_log_exp_and_others table set (covers
    # Exp/Ln/Copy/Identity) so exactly one LoadActFuncSet is emitted.
    from concourse import hw_specs as _hw
    _tabs = dict(_hw.get_activation_tables("gen3"))
    _pinned = {
        name: (fns if name == "natural_log_exp_and_others" else frozenset())
        for name, fns in _tabs.items()
    }
    _hw.get_activation_tables.cache_clear()
    _orig = _hw.get_activation_tables.__wrapped__

    def _patched(arch):
        if arch == "gen3":
            return _pinned
        return _orig(arch)

    _hw.get_activation_tables = _patched
    import concourse.bacc as _baccmod
    if hasattr(_baccmod, "get_activation_tables"):
        _baccmod.get_activation_tables = _patched

    nc = bacc.Bacc(None, target_bir_lowering=False)
    xt = nc.dram_tensor("xt", [128, L, M], bf16, kind="ExternalInput")
    xloc = nc.dram_tensor("xloc", [128, L, MS, D], bf16, kind="ExternalInput")
    wv = nc.dram_tensor("wv", [128, MS], f32, kind="ExternalInput")
    out = nc.dram_tensor("out_wlp", [128, L * MS], f32, kind="ExternalOutput")
    import os
    _dbg = os.environ.get("KDBG", "0") == "1"
    if _dbg:
        dden = nc.dram_tensor("dbg_den", [128, L * MS], f32, kind="ExternalOutput")
        dpos = nc.dram_tensor("dbg_pos", [128, L, MS], f32, kind="ExternalOutput")

    with ExitStack() as ctx:
        tc = ctx.enter_context(tile.TileContext(nc))
        singles = ctx.enter_context(tc.tile_pool(name="singles", bufs=1))
        xtp = ctx.enter_context(tc.tile_pool(name="xtp", bufs=1))
        x2p = ctx.enter_context(tc.tile_pool(name="x2p", bufs=2))
        zp = ctx.enter_context(tc.tile_pool(name="zp", bufs=2))
        ep = ctx.enter_context(tc.tile_pool(name="ep", bufs=10))
        # PSUM: sim tiles 2 banks x3 bufs + ztp 1 bank + persist 1 bank = 8
        simp = ctx.enter_context(tc.tile_pool(name="simp", bufs=6, space="PSUM"))
        ztpp = ctx.enter_context(tc.tile_pool(name="ztpp", bufs=1, space="PSUM"))
        perp = ctx.enter_context(tc.tile_pool(name="perp", bufs=1, space="PSUM"))

        identb = singles.tile([128, 128], bf16)
        make_identity(nc, identb[:])
        ones = singles.tile([128, 1], bf16)
        nc.vector.memset(ones[:], 1.0)
        onesf = singles.tile([128, 1], f32)
        nc.vector.memset(onesf[:], 1.0)

        w = singles.tile([128, MS], f32)
        nc.sync.dma_start(out=w[:], in_=wv[:, :])
        negclamp = singles.tile([128, 1], f32)
        nc.vector.memset(negclamp[:], -CLAMP)

        # persistent PSUM bank: ssq (cols 0..127, [l*32+j]) and den
        # (cols 128..143, [l*4+s])
        persist = perp.tile([128, 512], f32)

        XT = xtp.tile([128, L, M], bf16)
        for l in range(L):
            nc.gpsimd.dma_start(out=XT[:, l, :], in_=xt[:, l, :])
        XL = singles.tile([128, L, MS, D], bf16)
        nc.sync.dma_start(out=XL[:], in_=xloc[:, :, :, :])

        # ---- per-l prologue: norms and normalized local rows ----
        def prologue(l):
            x2 = x2p.tile([128, M], bf16)
            nc.vector.tensor_scalar(out=x2[:], in0=XT[:, l, :], scalar1=2.0,
                                    scalar2=None, op0=OP.pow)
            ssq = persist[:, l * NCH:(l + 1) * NCH]
            for j in range(NCH):
                nc.tensor.matmul(ssq[:, j:j + 1], x2[:, j * 128:(j + 1) * 128],
                                 ones[:], start=True, stop=True)
            lnssq = singles.tile([128, NCH], f32)
            nc.scalar.activation(out=lnssq[:], in_=ssq, func=AF.Ln)
            inv = singles.tile([128, NCH], f32)
            nc.scalar.activation(out=inv[:], in_=lnssq[:], func=AF.Exp,
                                 scale=-0.5)
            invT = singles.tile([128, NCH], f32)
            nc.vector.tensor_scalar(out=invT[:], in0=inv[:], scalar1=INV_T,
                                    scalar2=None, op0=OP.mult)
            base = singles.tile([128, NCH], f32)
            nc.scalar.activation(out=base[:], in_=inv[:], func=AF.Exp,
                                 scale=INV_T)
            # normalized local rows -> [D, 512] via PE transpose
            zs = zp.tile([128, MS, D], bf16)
            for s in range(MS):
                nc.vector.tensor_scalar(out=zs[:, s, :], in0=XL[:, l, s, :],
                                        scalar1=inv[:, s:s + 1], scalar2=None,
                                        op0=OP.mult)
            ztp = ztpp.tile([128, R], bf16)
            for s in range(MS):
                nc.tensor.transpose(ztp[:, s * 128:(s + 1) * 128],
                                    zs[:, s, :], identb[:])
            zT = zp.tile([128, R], bf16)
            nc.vector.tensor_copy(zT[:], ztp[:])
            invs.append(inv)
            invTs.append(invT)
            bases.append(base)
            zTs.append(zT)

        posE = singles.tile([128, L, MS], f32)

        # ---- main loop: sim -> exp (ACT/DVE) -> colsum ----
        # deferred colsums so PE lags the exp engines by a few items
        pend = []   # (l, [j...], E_tile)

        def flush_colsum(item):
            l, j, E, kind = item
            den = denps[:, l * MS:(l + 1) * MS]
            for s in range(MS):
                lhs = E[:, s * 128:(s + 1) * 128]
                if kind == "D":
                    lhs = lhs.bitcast(f32)
                nc.tensor.matmul(
                    den[:, s:s + 1], lhs,
                    onesf[:] if kind == "D" else ones[:],
                    start=(j == 0), stop=(j == NCH - 1),
                    skip_group_check=True)
            if j == NCH - 1:
                tail_l(l)

        def extras(l, j, E):
            # self/pos chunks are forced to ACT, so E is bf16 here
            if j < MS:
                # clamp the self-similarity diagonal sub-square
                cols = slice(j * 128, (j + 1) * 128)
                nc.gpsimd.tensor_scalar(
                    out=E[:, cols], in0=E[:, cols], scalar1=CLAMP,
                    scalar2=None, op0=OP.min)
            if 16 <= j < 16 + MS:
                # positive-pair diagonal (stt is DVE-only on real HW)
                s = j - 16
                cols = slice(s * 128, (s + 1) * 128)
                junk = junkp.tile([128, 128], bf16)
                nc.vector.scalar_tensor_tensor(
                    out=junk[:], in0=E[:, cols], scalar=1.0,
                    in1=identb[:], op0=OP.mult, op1=OP.mult,
                    accum_out=posE[:, l, s:s + 1])

        def tail_l(l):
            den = denps[:, l * MS:(l + 1) * MS]
            logd = singles.tile([128, MS], f32)
            nc.scalar.activation(out=logd[:], in_=den, func=AF.Ln,
                                 bias=negclamp[:])
            posln = singles.tile([128, MS], f32)
            nc.scalar.activation(out=posln[:], in_=posE[:, l, :], func=AF.Ln)
            posT = singles.tile([128, MS], f32)
            nc.gpsimd.tensor_tensor(out=posT[:], in0=posln[:],
                                    in1=invs[l][:, 0:MS], op=OP.mult)
            lp = singles.tile([128, MS], f32)
            nc.gpsimd.tensor_tensor(out=lp[:], in0=logd[:], in1=posT[:],
                                    op=OP.subtract)
            nc.gpsimd.tensor_tensor(out=out_sb[:, l, :], in0=lp[:], in1=w[:],
                                    op=OP.mult)

        prologue(0)
        for l in range(L):
            inv = state[(l, "inv")]
            invT = state[(l, "invT")]
            An = state[(l, "An")]
            zT = state[(l, "zT")]
            invs[l] = inv
            for kind, j in SCHED:
                if l + 1 < L:
                    for pos, fn in PIPE:
                        if j == pos:
                            fn(l + 1)
                sim = simp.tile([128, R], f32)
                nc.tensor.matmul(sim[:], XTs[l][:, j * 128:(j + 1) * 128],
                                 zT[:], start=True, stop=True)
                if kind == "A":
                    E = ep.tile([128, R], bf16)
                    nc.scalar.activation(out=E[:], in_=sim[:], func=AF.Exp,
                                         scale=invT[:, j:j + 1])
                    extras(l, j, E)
                else:
                    E = ep.tile([128, R], i32)
                    nc.vector.tensor_scalar(
                        out=E[:], in0=sim[:], scalar1=An[:, j:j + 1],
                        scalar2=SCH_B, op0=OP.mult, op1=OP.add)
                pend.append((l, j, E, kind))
                if len(pend) > 7:
                    flush_colsum(pend.pop(0))
        while pend:
            flush_colsum(pend.pop(0))

        # ---- tail ----
        out_sb = singles.tile([128, L, MS], f32)
        for l in range(L):
            den = persist[:, 128 + l * MS:128 + (l + 1) * MS]
            logd = singles.tile([128, MS], f32)
            nc.scalar.activation(out=logd[:], in_=den, func=AF.Ln, bias=-CLAMP)
            posln = singles.tile([128, MS], f32)
            nc.scalar.activation(out=posln[:], in_=posE[:, l, :], func=AF.Ln)
            posT = singles.tile([128, MS], f32)
            nc.vector.tensor_tensor(out=posT[:], in0=posln[:],
                                    in1=invs[l][:, 0:MS], op=OP.mult)
            lp = singles.tile([128, MS], f32)
            nc.vector.tensor_tensor(out=lp[:], in0=logd[:], in1=posT[:],
                                    op=OP.subtract)
            nc.vector.tensor_tensor(out=out_sb[:, l, :], in0=lp[:], in1=w[:],
                                    op=OP.mult)
        nc.sync.dma_start(out=out[:, :], in_=out_sb[:].rearrange("p l s -> p (l s)"))

    nc.finalize()
    _built = nc
    return nc


def _in_maps(emb_i, emb_j, joint_valid):
    import ml_dtypes
    emb_i = np.asarray(emb_i, dtype=np.float32)
    emb_j = np.asarray(emb_j, dtype=np.float32)
    jv = np.asarray(joint_valid, dtype=np.float32).reshape(-1)
    reps = np.concatenate(
        [emb_i.reshape(L, N, D), emb_j.reshape(L, N, D)], axis=1)  # [L, M, D]
    maps = []
    for c in range(NCORES):
        idx = (np.arange(M) + c * R) % M
        rolled = reps[:, idx, :]                       # [L, M, D]
        xt = np.ascontiguousarray(
            rolled.transpose(2, 0, 1)).astype(ml_dtypes.bfloat16)
        xl = np.ascontiguousarray(
            rolled[:, :R, :].reshape(L, MS, 128, D).transpose(2, 0, 1, 3)
        ).astype(ml_dtypes.bfloat16)
        w = jv[(np.arange(R) + c * R) % N].reshape(MS, 128).T  # [128, MS]
        maps.append({"xt": xt, "xloc": xl, "wv": np.ascontiguousarray(w)})
    return maps, jv


def _combine(results, jv):
    tot = 0.0
    for r in results:
        tot += float(r["out_wlp"].astype(np.float64).sum())
    return np.float32(tot / (2.0 * float(jv.sum())))


def kernel(emb_i, emb_j, joint_valid):
    from concourse.bass_utils import run_bass_kernel_spmd

    nc = _build()
    maps, jv = _in_maps(emb_i, emb_j, joint_valid)
    res = run_bass_kernel_spmd(nc, maps, core_ids=list(range(NCORES)))
    return _combine(res.results, jv)


def run_traced(inputs, trace_cores=None):
    """test.py helper: same run but with NTFF tracing enabled."""
    from concourse.bass_utils import run_bass_kernel_spmd

    nc = _build()
    maps, jv = _in_maps(**inputs)
    res = run_bass_kernel_spmd(
        nc, maps, core_ids=list(range(NCORES)), trace=True,
        trace_cores=trace_cores if trace_cores is not None else list(range(NCORES)))
    res.loss = _combine(res.results, jv)
    return res
